# revision 11
# baseline (speedup 1.0000x reference)
"""VQ codebook nearest-neighbor lookup on 8 TRN2 NeuronCores.

reference math: argmin_k ||x_n - c_k||^2 ; quantized = weight[argmin].
Codebook rows are L2-normalized (||c_k|| == 1 up to ~1e-7), so
argmin dist == argmax (x . c_k) up to a c_sq bias ~1e-7 -- far below every
noise margin here; the host re-pick uses exact distances anyway.

Device side (data parallel over N: 8 shards of 4096 rows, codebook
replicated). The first max-fold level runs INSIDE the PE via
max(a,b) = (a+b)/2 + |a-b|/2: the host ships u = (c_k + c_{k+4096})/2 and
v = (c_k - c_{k+4096})/2 codebooks (fp8 e4m3, transposed); per 128-row
tile the device computes V = x@v into PSUM with DoubleRow fp8 matmuls,
ACT applies |.| in place, and the U = x@u matmuls accumulate on top
(start=False), leaving Z[q] = max(s_q, s_{q+4096}) -- only 4 PSUM
kilochunks per tile instead of 8. ACT then drains Z0 to SBUF fp16 and DVE
max-eats Z1..Z3 against it, folds 1024->128, and MAX8 + FIND_INDEX8 top-8.
Work is emitted in half-tile "micro" steps (2048 pair-cols) so the 16KB
PSUM holds two micros and the U-accumulate pipelines one micro behind the
V pass. Engine busy per core/tile: ACT ~5.0us (4 abs + 1 drain), DVE
~4.4us, PE ~3.6us -> ~5us/tile, ~165us total (CoreSim cost model; 216us
for the previous all-ACT/DVE drain pipeline, 594us for the f32r/full-
argmax baseline).

fp8 score noise is sigma ~0.057 (two fp8 matmuls per Z), absorbed by the
host-side exact re-pick: expand the fold-mates of every slot within
MARGIN of the best (the top slot always) and rescore exactly. All rows
sharing a slot rescore against the same 64 codebook rows, so the rescore
runs as <=128 small GEMMs, then argmin and weight[best] gather.
"""

import os
import sys

for _p in (
    "/opt/trn_rl_repo",
    "/root/.axon_site",
    "/root/.axon_site/_ro/trn_rl_repo",
    "/root/.axon_site/_ro/pypackages",
):
    if os.path.isdir(_p) and _p not in sys.path:
        sys.path.append(_p)

from contextlib import ExitStack

import numpy as np

import concourse.bass as bass
import concourse.tile as tile
from concourse import bacc, bass_utils, mybir

N_CORES = 8
N, K, D = 32768, 8192, 512
NS = N // N_CORES  # rows per core
P = 128
NT = NS // P  # row-tiles per core
KP = K // 2  # pair-columns after the in-PE max fold: 4096
F8 = mybir.dt.float8e4
F16 = mybir.dt.float16
F32 = mybir.dt.float32
U16 = mybir.dt.uint16

KC = 1024  # PSUM kilochunk width
WTC = 2048  # codebook tile width in SBUF (= pair-cols per micro step)
FOLD = 64
FW = K // FOLD  # folded row width: 128
MARGIN = 0.40  # fp8 score-noise margin for host re-pick (sigma ~0.057)
DR = mybir.MatmulPerfMode.DoubleRow
MAX = mybir.AluOpType.max
ABS = mybir.ActivationFunctionType.Abs


def _build_program():
    nc = bacc.Bacc(
        "TRN2", target_bir_lowering=False, debug=False, enable_asserts=False,
        num_devices=N_CORES,
    )
    # x row-tiles pre-swizzled on host to DoubleRow lhsT layout:
    # xt[i*128+p, t, i2, c] = x[i*128+c, t*256+i2*128+p]
    xt_d = nc.dram_tensor("xt", [NT * P, 2, 2, P], F8, kind="ExternalInput").ap()
    wu_d = nc.dram_tensor("wu", [D, KP], F8, kind="ExternalInput").ap()
    wv_d = nc.dram_tensor("wv", [D, KP], F8, kind="ExternalInput").ap()
    # per row: [0:8] = top-8 folded-slot maxes (fp16), [8:16] = slot ids (u16)
    top_d = nc.dram_tensor("top", [NS, 16], F16, kind="ExternalOutput").ap()

    with tile.TileContext(nc) as tc, ExitStack() as ctx:
        wt_pool = ctx.enter_context(tc.tile_pool(name="wt", bufs=1))
        x_pool = ctx.enter_context(tc.tile_pool(name="x", bufs=4))
        ps_pool = ctx.enter_context(tc.tile_pool(name="ps", bufs=2, space="PSUM"))
        s_pool = ctx.enter_context(tc.tile_pool(name="s", bufs=2))
        e_pool = ctx.enter_context(tc.tile_pool(name="e", bufs=2))
        m_pool = ctx.enter_context(tc.tile_pool(name="m", bufs=2))
        o_pool = ctx.enter_context(tc.tile_pool(name="o", bufs=3))

        xs = [None] * NT

        def load_x(i):
            xt = x_pool.tile([P, 2, 2, P], F8, name="X", tag="X")
            nc.sync.dma_start(out=xt[:, :, :, :], in_=xt_d[i * P : (i + 1) * P])
            xs[i] = xt

        load_x(0)
        load_x(1)

        # u/v codebooks in SBUF as [128, 2, WTC] fp8 tiles; dim1 is the
        # DoubleRow sub-row pair: global contraction row d = t*256 + i2*128 + p.
        # wv streams first (sync+scalar queues), wu behind it on gpsimd.
        wu = [[None, None], [None, None]]
        wv = [[None, None], [None, None]]
        for c in range(2):
            for t in range(2):
                # keep the ACT (scalar) queue DMA-free: its DMAs block the
                # ACT engine, which is the pipeline's critical resource
                for w, ltr, store, q in (
                    (wv, "v", wv_d, nc.sync),
                    (wu, "u", wu_d, nc.gpsimd),
                ):
                    wtile = wt_pool.tile(
                        [P, 2, WTC], F8,
                        name=f"w{ltr}_{t}_{c}",
                        tag=f"w{ltr}_{t}_{c}",
                    )
                    for i2 in range(2):
                        q.dma_start(
                            out=wtile[:, i2, :],
                            in_=store[t * 256 + i2 * 128 : t * 256 + (i2 + 1) * 128,
                                      c * WTC : (c + 1) * WTC],
                        )
                    w[t][c] = wtile

        def mm(ps, jj, x_tile, w, h, start):
            """4 DoubleRow matmuls filling kilochunk jj of the [P, 2048]
            PSUM tile `ps` (pair-cols [h*2048 + jj*1024, +1024))."""
            for r in range(2):
                woff = jj * KC + r * 512
                for t in range(2):
                    nc.tensor.matmul(
                        ps[:, woff : woff + 512],
                        lhsT=x_tile[:, t, :, :],
                        rhs=w[t][h][:, :, woff : woff + 512],
                        start=start and (t == 0),
                        stop=(t == 1),
                        perf_mode=DR,
                    )

        # per-tile consumer state: S (drained Z0), E0..E2 chain
        st = [dict() for _ in range(NT)]
        psm = [None] * (2 * NT)  # per-micro [P, 2048] PSUM tile

        def consume(m):
            """Emit U-accumulate + Z consumers for micro m (runs one micro
            later than m's V/abs pass)."""
            i, h = divmod(m, 2)
            for jj in range(2):
                mm(psm[m], jj, xs[i], wu, h, start=False)
            s = st[i]
            if h == 0:
                s["S"] = s_pool.tile([P, KC], F16, name="S", tag="S")
                nc.scalar.copy(out=s["S"][:], in_=psm[m][:, 0:KC])
                s["E0"] = e_pool.tile([P, KC], F16, name="E0", tag="E0")
                nc.vector.tensor_tensor(out=s["E0"][:], in0=psm[m][:, KC : 2 * KC],
                                        in1=s["S"][:], op=MAX)
            else:
                E1 = e_pool.tile([P, KC], F16, name="E1", tag="E1")
                nc.vector.tensor_tensor(out=E1[:], in0=psm[m][:, 0:KC],
                                        in1=s["E0"][:], op=MAX)
                E2 = e_pool.tile([P, KC], F16, name="E2", tag="E2")
                nc.vector.tensor_tensor(out=E2[:], in0=psm[m][:, KC : 2 * KC],
                                        in1=E1[:], op=MAX)
                F1 = m_pool.tile([P, 512], F16, name="F1", tag="F1")
                nc.vector.tensor_tensor(out=F1[:], in0=E2[:, 0:512],
                                        in1=E2[:, 512:1024], op=MAX)
                F2 = m_pool.tile([P, 256], F16, name="F2", tag="F2")
                nc.vector.tensor_tensor(out=F2[:], in0=F1[:, 0:256],
                                        in1=F1[:, 256:512], op=MAX)
                Fm = m_pool.tile([P, FW], F16, name="Fm", tag="Fm")
                nc.vector.tensor_tensor(out=Fm[:], in0=F2[:, 0:FW],
                                        in1=F2[:, FW : 2 * FW], op=MAX)
                o = o_pool.tile([P, 16], F16, name="o", tag="o")
                nc.vector.max(out=o[:, 0:8], in_=Fm[:])
                nc.vector.max_index(out=o[:, 8:16].bitcast(U16),
                                    in_max=o[:, 0:8], in_values=Fm[:])
                nc.sync.dma_start(out=top_d[i * P : (i + 1) * P, :], in_=o[:])

        for m in range(2 * NT):
            i, h = divmod(m, 2)
            if m > 0:
                consume(m - 1)
            if h == 0 and i + 2 < NT:
                load_x(i + 2)
            # V pass for micro m + one wide in-place |V| on ACT
            psm[m] = ps_pool.tile([P, 2 * KC], F32, name="ps", tag="ps")
            for jj in range(2):
                mm(psm[m], jj, xs[i], wv, h, start=True)
            nc.scalar.activation(out=psm[m][:], in_=psm[m][:], func=ABS)
        consume(2 * NT - 1)

    nc.compile()
    return nc


_NC = None
_JIT = None  # (sharded_fn, in_names, out_names, out_avals, n_params)
last_exec_time_ns = None


def _run_cached(nc, in_maps):
    """Multi-core dispatch equivalent to bass2jax.run_bass_via_pjrt, but with
    the jitted executable cached so repeat kernel() calls skip recompilation."""
    global _JIT
    import jax
    import numpy as _np
    from jax.experimental.shard_map import shard_map
    from jax.sharding import Mesh, PartitionSpec

    from concourse import bass2jax, mybir as _mb
    from concourse.bass2jax import _bass_exec_p, install_neuronx_cc_hook

    if _JIT is None:
        install_neuronx_cc_hook()
        partition_name = nc.partition_id_tensor.name if nc.partition_id_tensor else None
        in_names, out_names, out_avals = [], [], []
        for alloc in nc.m.functions[0].allocations:
            if not isinstance(alloc, _mb.MemoryLocationSet):
                continue
            name = alloc.memorylocations[0].name
            if alloc.kind == "ExternalInput":
                if name != partition_name:
                    in_names.append(name)
            elif alloc.kind == "ExternalOutput":
                out_names.append(name)
                out_avals.append(
                    jax.core.ShapedArray(
                        tuple(alloc.tensor_shape), _mb.dt.np(alloc.dtype)
                    )
                )
        n_params = len(in_names)
        all_in_names = list(in_names) + list(out_names)
        if partition_name is not None:
            all_in_names.append(partition_name)
        donate = tuple(range(n_params, n_params + len(out_names)))

        def _body(*args):
            operands = list(args)
            if partition_name is not None:
                operands.append(bass2jax.partition_id_tensor())
            return tuple(
                _bass_exec_p.bind(
                    *operands,
                    out_avals=tuple(out_avals),
                    in_names=tuple(all_in_names),
                    out_names=tuple(out_names),
                    lowering_input_output_aliases=(),
                    sim_require_finite=True,
                    sim_require_nnan=True,
                    nc=nc,
                )
            )

        devices = jax.devices()[:N_CORES]
        mesh = Mesh(_np.asarray(devices), ("core",))
        specs_in = (PartitionSpec("core"),) * (n_params + len(out_names))
        specs_out = (PartitionSpec("core"),) * len(out_names)
        sharded = jax.jit(
            shard_map(
                _body, mesh=mesh, in_specs=specs_in, out_specs=specs_out,
                check_rep=False,
            ),
            donate_argnums=donate,
            keep_unused=True,
        )
        _JIT = (sharded, in_names, out_names, out_avals, n_params)

    sharded, in_names, out_names, out_avals, n_params = _JIT
    concat_in = [
        np.concatenate([np.asarray(m[name]) for m in in_maps], axis=0)
        for name in in_names
    ]
    concat_zeros = [
        np.zeros((N_CORES * a.shape[0], *a.shape[1:]), a.dtype) for a in out_avals
    ]
    out_arrs = sharded(*concat_in, *concat_zeros)
    return [
        {
            name: np.asarray(out_arrs[i]).reshape(N_CORES, *out_avals[i].shape)[c]
            for i, name in enumerate(out_names)
        }
        for c in range(N_CORES)
    ]


def kernel(x: np.ndarray, weight: np.ndarray) -> np.ndarray:
    global _NC, last_exec_time_ns
    assert x.shape == (N, D) and weight.shape == (K, D)
    if _NC is None:
        _NC = _build_program()

    e4 = mybir.dt.np(F8)
    x = np.ascontiguousarray(x, dtype=np.float32)
    weight = np.ascontiguousarray(weight, dtype=np.float32)
    xt8 = np.ascontiguousarray(x.T).astype(e4)          # [D, N]
    u = (weight[:KP] + weight[KP:]) * 0.5               # [KP, D]
    v = (weight[:KP] - weight[KP:]) * 0.5
    wu8 = np.ascontiguousarray(u.T).astype(e4)          # [D, KP]
    wv8 = np.ascontiguousarray(v.T).astype(e4)
    in_maps = []
    for c in range(N_CORES):
        xc = xt8[:, c * NS : (c + 1) * NS]              # [D, NS]
        # [t, i2, p, i, cc] -> [i, p, t, i2, cc]: DoubleRow lhsT tile layout
        xh = np.ascontiguousarray(
            xc.reshape(2, 2, P, NT, P).transpose(3, 2, 0, 1, 4)
        ).reshape(NT * P, 2, 2, P)
        in_maps.append({"xt": xh, "wu": wu8, "wv": wv8})

    results = None
    if os.environ.get("KERNEL_TRACE"):
        try:
            res = bass_utils.run_bass_kernel_spmd(
                _NC, in_maps, core_ids=list(range(N_CORES)), trace=True,
            )
            last_exec_time_ns = res.exec_time_ns
            results = res.results
        except Exception:
            results = None  # no NTFF profiling hook in this env; run untraced
    if results is None:
        results = _run_cached(_NC, in_maps)

    top = np.concatenate(
        [results[i]["top"] for i in range(N_CORES)], axis=0
    )                                                    # [N, 16] fp16-typed
    topv = top[:, 0:8].astype(np.float32)                # [N, 8] folded maxes
    slots = (
        np.ascontiguousarray(top[:, 8:16]).view(np.uint16).astype(np.int64)
    )                                                    # [N, 8] folded slot ids

    # Expand fold-mates of the top slot plus every slot within MARGIN, then
    # pick by exact distance. Slot s covers codebook ids {s + FW*m}; all rows
    # sharing a slot rescore against the same 64 codebook rows, so the
    # rescore runs as <=FW small GEMMs instead of millions of gathered dots.
    assert (slots < FW).all()
    in_margin = topv >= (topv[:, 0:1] - MARGIN)
    in_margin[:, 0] = True
    r_pairs, p_pairs = np.nonzero(in_margin)
    s_pairs = slots[r_pairs, p_pairs]
    W3 = np.ascontiguousarray(weight.reshape(FOLD, FW, D).transpose(1, 0, 2))
    c_sq = np.einsum("kd,kd->k", weight, weight)
    C3 = np.ascontiguousarray(c_sq.reshape(FOLD, FW).T).astype(np.float64)
    best = np.full(N, -1, dtype=np.int64)
    best_d = np.full(N, np.inf)
    order = np.argsort(s_pairs, kind="stable")
    r_o, s_o = r_pairs[order], s_pairs[order]
    bounds = np.searchsorted(s_o, np.arange(FW + 1))
    for j in range(FW):
        lo, hi = bounds[j], bounds[j + 1]
        if lo == hi:
            continue
        rows = r_o[lo:hi]
        sc = x[rows] @ W3[j].T                     # [b, FOLD] fp32 GEMM
        d = C3[j][None, :] - 2.0 * sc.astype(np.float64)
        bi = np.argmin(d, axis=1)                  # first min -> smallest mate
        dmin = d[np.arange(len(rows)), bi]
        cmin = j + FW * bi
        sel = (dmin < best_d[rows]) | ((dmin == best_d[rows]) & (cmin < best[rows]))
        best[rows[sel]] = cmin[sel]
        best_d[rows[sel]] = dmin[sel]

    return weight[best]


# revision 28
# speedup vs baseline: 1.4638x; 1.4638x over previous
"""VQ codebook nearest-neighbor lookup on 8 TRN2 NeuronCores.

reference math: argmin_k ||x_n - c_k||^2 ; quantized = weight[argmin].
Codebook rows are L2-normalized (||c_k|| == 1 up to ~1e-7), so
argmin dist == argmax (x . c_k) up to a c_sq bias ~1e-7 -- far below every
noise margin here; the host re-pick uses exact distances anyway.

Device side (data parallel over N: 8 shards of 4096 rows, codebook
replicated). The first max-fold level runs INSIDE the PE via
max(a,b) = (a+b)/2 + |a-b|/2: the host ships u = (c_k + c_{k+4096})/2 and
v = (c_k - c_{k+4096})/2 codebooks (fp8 e4m3, transposed); per 128-row
tile the device computes V = x@v into PSUM with DoubleRow fp8 matmuls,
ACT applies |.| in place, and the U = x@u matmuls accumulate on top
(start=False), leaving Z[q] = max(s_q, s_{q+4096}) -- only 4 PSUM
kilochunks per tile instead of 8. ACT then drains Z0 to SBUF fp16 and DVE
max-eats Z1..Z3 against it, folds 1024->128, and MAX8 + FIND_INDEX8 top-8.
Work is emitted in half-tile "micro" steps (2048 pair-cols) so the 16KB
PSUM holds two micros and the U-accumulate pipelines one micro behind the
V pass. Engine busy per core/tile: ACT ~5.0us (4 abs + 1 drain), DVE
~4.4us, PE ~3.6us -> ~5us/tile, ~165us total (CoreSim cost model; 216us
for the previous all-ACT/DVE drain pipeline, 594us for the f32r/full-
argmax baseline).

fp8 score noise is sigma ~0.057 (two fp8 matmuls per Z), absorbed by the
host-side exact re-pick: expand the fold-mates of every slot within
MARGIN of the best (the top slot always) and rescore exactly. All rows
sharing a slot rescore against the same 64 codebook rows, so the rescore
runs as <=128 small GEMMs, then argmin and weight[best] gather.
"""

import os
import sys

for _p in (
    "/opt/trn_rl_repo",
    "/root/.axon_site",
    "/root/.axon_site/_ro/trn_rl_repo",
    "/root/.axon_site/_ro/pypackages",
):
    if os.path.isdir(_p) and _p not in sys.path:
        sys.path.append(_p)

from contextlib import ExitStack

import numpy as np

import concourse.bass as bass
import concourse.tile as tile
from concourse import bacc, bass_utils, mybir

N_CORES = 8
N, K, D = 32768, 8192, 512
NS = N // N_CORES  # rows per core
P = 128
NT = NS // P  # row-tiles per core
KP = K // 2  # pair-columns after the in-PE max fold: 4096
F8 = mybir.dt.float8e4
F16 = mybir.dt.float16
F32 = mybir.dt.float32
U16 = mybir.dt.uint16

KC = 1024  # PSUM kilochunk width
WTC = 2048  # codebook tile width in SBUF (= pair-cols per micro step)
FOLD = 64
FW = K // FOLD  # folded row width: 128
MARGIN = 0.40  # fp8 score-noise margin for host re-pick (sigma ~0.057)
DR = mybir.MatmulPerfMode.DoubleRow
MAX = mybir.AluOpType.max
ABS = mybir.ActivationFunctionType.Abs
# every CONST_EVERY-th tile replaces the ACT drain of Z0 with a DVE
# max-against--inf eat, shaving the critical ACT engine at DVE's expense
CONST_EVERY = int(os.environ.get("KERNEL_CONST_EVERY", "3"))


def _build_program():
    nc = bacc.Bacc(
        "TRN2", target_bir_lowering=False, debug=False, enable_asserts=False,
        num_devices=N_CORES,
    )
    # x row-tiles pre-swizzled on host to DoubleRow lhsT layout:
    # xt[i*128+p, t, i2, c] = x[i*128+c, t*256+i2*128+p]
    xt_d = nc.dram_tensor("xt", [NT * P, 2, 2, P], F8, kind="ExternalInput").ap()
    wu_d = nc.dram_tensor("wu", [D, KP], F8, kind="ExternalInput").ap()
    wv_d = nc.dram_tensor("wv", [D, KP], F8, kind="ExternalInput").ap()
    # per row: all FW folded-slot maxes (fp16); the host picks candidates
    top_d = nc.dram_tensor("top", [NS, FW], F16, kind="ExternalOutput").ap()

    with tile.TileContext(nc) as tc, ExitStack() as ctx:
        wt_pool = ctx.enter_context(tc.tile_pool(name="wt", bufs=1))
        x_pool = ctx.enter_context(tc.tile_pool(name="x", bufs=4))
        ps_pool = ctx.enter_context(tc.tile_pool(name="ps", bufs=4, space="PSUM"))
        s_pool = ctx.enter_context(tc.tile_pool(name="s", bufs=2))
        e_pool = ctx.enter_context(tc.tile_pool(name="e", bufs=2))
        m_pool = ctx.enter_context(tc.tile_pool(name="m", bufs=2))
        o_pool = ctx.enter_context(tc.tile_pool(name="o", bufs=3))

        xs = [None] * NT
        neg = wt_pool.tile([P, KC], F16, name="neg", tag="neg")
        nc.gpsimd.memset(neg[:], -1000.0)

        def load_x(i):
            xt = x_pool.tile([P, 2, 2, P], F8, name="X", tag="X")
            nc.sync.dma_start(out=xt[:, :, :, :], in_=xt_d[i * P : (i + 1) * P])
            xs[i] = xt

        load_x(0)
        load_x(1)

        # u/v codebooks in SBUF as [128, 2, WTC] fp8 tiles; dim1 is the
        # DoubleRow sub-row pair: global contraction row d = t*256 + i2*128 + p.
        # wv streams first (sync+scalar queues), wu behind it on gpsimd.
        wu = [[None, None], [None, None]]
        wv = [[None, None], [None, None]]
        for c in range(2):
            for t in range(2):
                # wv splits across sync+scalar so the first V matmuls (and the
                # first ACT abs behind them) start as early as possible; the
                # scalar-queue DMAs block the ACT engine but only during fill,
                # before ACT's first abs is runnable anyway
                for w, ltr, store, q in (
                    (wv, "v", wv_d, nc.sync if c == 0 else nc.scalar),
                    (wu, "u", wu_d, nc.gpsimd),
                ):
                    wtile = wt_pool.tile(
                        [P, 2, WTC], F8,
                        name=f"w{ltr}_{t}_{c}",
                        tag=f"w{ltr}_{t}_{c}",
                    )
                    for i2 in range(2):
                        q.dma_start(
                            out=wtile[:, i2, :],
                            in_=store[t * 256 + i2 * 128 : t * 256 + (i2 + 1) * 128,
                                      c * WTC : (c + 1) * WTC],
                        )
                    w[t][c] = wtile

        def mm(ps, jj, x_tile, w, h, start):
            """4 DoubleRow matmuls filling kilochunk `ps` (pair-cols
            [h*2048 + jj*1024, +1024)) from weight tiles w[t][h]."""
            for r in range(2):
                off = r * 512
                woff = jj * KC + off
                for t in range(2):
                    nc.tensor.matmul(
                        ps[:, off : off + 512],
                        lhsT=x_tile[:, t, :, :],
                        rhs=w[t][h][:, :, woff : woff + 512],
                        start=start and (t == 0),
                        stop=(t == 1),
                        perf_mode=DR,
                    )

        # per-tile consumer state: S (drained Z0), E0..E2 chain
        st = [dict() for _ in range(NT)]
        psm = [None] * (2 * NT)  # per-micro [P, 2048] PSUM tile

        def consume(m):
            """Emit U-accumulate + Z consumers for micro m (runs one micro
            later than m's V/abs pass)."""
            i, h = divmod(m, 2)
            for jj in range(2):
                mm(psm[m][jj], jj, xs[i], wu, h, start=False)
            s = st[i]
            if h == 0:
                if CONST_EVERY and i % CONST_EVERY == CONST_EVERY - 1:
                    Ec = e_pool.tile([P, KC], F16, name="Ec", tag="Ec")
                    nc.vector.tensor_tensor(out=Ec[:], in0=psm[m][0][:],
                                            in1=neg[:], op=MAX)
                else:
                    Ec = s_pool.tile([P, KC], F16, name="S", tag="S")
                    nc.scalar.copy(out=Ec[:], in_=psm[m][0][:])
                s["E0"] = e_pool.tile([P, KC], F16, name="E0", tag="E0")
                nc.vector.tensor_tensor(out=s["E0"][:], in0=psm[m][1][:],
                                        in1=Ec[:], op=MAX)
            else:
                E1 = e_pool.tile([P, KC], F16, name="E1", tag="E1")
                nc.vector.tensor_tensor(out=E1[:], in0=psm[m][0][:],
                                        in1=s["E0"][:], op=MAX)
                E2 = e_pool.tile([P, KC], F16, name="E2", tag="E2")
                nc.vector.tensor_tensor(out=E2[:], in0=psm[m][1][:],
                                        in1=E1[:], op=MAX)
                F1 = m_pool.tile([P, 512], F16, name="F1", tag="F1")
                nc.vector.tensor_tensor(out=F1[:], in0=E2[:, 0:512],
                                        in1=E2[:, 512:1024], op=MAX)
                F2 = m_pool.tile([P, 256], F16, name="F2", tag="F2")
                nc.vector.tensor_tensor(out=F2[:], in0=F1[:, 0:256],
                                        in1=F1[:, 256:512], op=MAX)
                Fm = o_pool.tile([P, FW], F16, name="Fm", tag="Fm")
                nc.vector.tensor_tensor(out=Fm[:], in0=F2[:, 0:FW],
                                        in1=F2[:, FW : 2 * FW], op=MAX)
                # ship the whole folded row; the host top-picks over all 128
                # slots (saves MAX8 + FIND_INDEX8 on the critical DVE engine)
                nc.sync.dma_start(out=top_d[i * P : (i + 1) * P, :], in_=Fm[:])

        for m in range(2 * NT):
            i, h = divmod(m, 2)
            if m > 0:
                consume(m - 1)
            if h == 0 and i + 2 < NT:
                load_x(i + 2)
            # V pass for micro m + in-place |V| on ACT
            psm[m] = [ps_pool.tile([P, KC], F32, name="ps", tag="ps")
                      for _ in range(2)]
            for jj in range(2):
                mm(psm[m][jj], jj, xs[i], wv, h, start=True)
            for jj in range(2):
                nc.scalar.activation(out=psm[m][jj][:], in_=psm[m][jj][:],
                                     func=ABS)
        consume(2 * NT - 1)

    nc.compile()
    return nc


_NC = None
_JIT = None  # (sharded_fn, in_names, out_names, out_avals, n_params)
last_exec_time_ns = None


def _run_cached(nc, in_maps):
    """Multi-core dispatch equivalent to bass2jax.run_bass_via_pjrt, but with
    the jitted executable cached so repeat kernel() calls skip recompilation."""
    global _JIT
    import jax
    import numpy as _np
    from jax.experimental.shard_map import shard_map
    from jax.sharding import Mesh, PartitionSpec

    from concourse import bass2jax, mybir as _mb
    from concourse.bass2jax import _bass_exec_p, install_neuronx_cc_hook

    if _JIT is None:
        install_neuronx_cc_hook()
        partition_name = nc.partition_id_tensor.name if nc.partition_id_tensor else None
        in_names, out_names, out_avals = [], [], []
        for alloc in nc.m.functions[0].allocations:
            if not isinstance(alloc, _mb.MemoryLocationSet):
                continue
            name = alloc.memorylocations[0].name
            if alloc.kind == "ExternalInput":
                if name != partition_name:
                    in_names.append(name)
            elif alloc.kind == "ExternalOutput":
                out_names.append(name)
                out_avals.append(
                    jax.core.ShapedArray(
                        tuple(alloc.tensor_shape), _mb.dt.np(alloc.dtype)
                    )
                )
        n_params = len(in_names)
        all_in_names = list(in_names) + list(out_names)
        if partition_name is not None:
            all_in_names.append(partition_name)
        donate = tuple(range(n_params, n_params + len(out_names)))

        def _body(*args):
            operands = list(args)
            if partition_name is not None:
                operands.append(bass2jax.partition_id_tensor())
            return tuple(
                _bass_exec_p.bind(
                    *operands,
                    out_avals=tuple(out_avals),
                    in_names=tuple(all_in_names),
                    out_names=tuple(out_names),
                    lowering_input_output_aliases=(),
                    sim_require_finite=True,
                    sim_require_nnan=True,
                    nc=nc,
                )
            )

        devices = jax.devices()[:N_CORES]
        mesh = Mesh(_np.asarray(devices), ("core",))
        specs_in = (PartitionSpec("core"),) * (n_params + len(out_names))
        specs_out = (PartitionSpec("core"),) * len(out_names)
        sharded = jax.jit(
            shard_map(
                _body, mesh=mesh, in_specs=specs_in, out_specs=specs_out,
                check_rep=False,
            ),
            donate_argnums=donate,
            keep_unused=True,
        )
        _JIT = (sharded, in_names, out_names, out_avals, n_params)

    sharded, in_names, out_names, out_avals, n_params = _JIT
    concat_in = [
        np.concatenate([np.asarray(m[name]) for m in in_maps], axis=0)
        for name in in_names
    ]
    concat_zeros = [
        np.zeros((N_CORES * a.shape[0], *a.shape[1:]), a.dtype) for a in out_avals
    ]
    out_arrs = sharded(*concat_in, *concat_zeros)
    return [
        {
            name: np.asarray(out_arrs[i]).reshape(N_CORES, *out_avals[i].shape)[c]
            for i, name in enumerate(out_names)
        }
        for c in range(N_CORES)
    ]


def kernel(x: np.ndarray, weight: np.ndarray) -> np.ndarray:
    global _NC, last_exec_time_ns
    assert x.shape == (N, D) and weight.shape == (K, D)
    if _NC is None:
        _NC = _build_program()

    e4 = mybir.dt.np(F8)
    x = np.ascontiguousarray(x, dtype=np.float32)
    weight = np.ascontiguousarray(weight, dtype=np.float32)
    xt8 = np.ascontiguousarray(x.T).astype(e4)          # [D, N]
    u = (weight[:KP] + weight[KP:]) * 0.5               # [KP, D]
    v = (weight[:KP] - weight[KP:]) * 0.5
    wu8 = np.ascontiguousarray(u.T).astype(e4)          # [D, KP]
    wv8 = np.ascontiguousarray(v.T).astype(e4)
    in_maps = []
    for c in range(N_CORES):
        xc = xt8[:, c * NS : (c + 1) * NS]              # [D, NS]
        # [t, i2, p, i, cc] -> [i, p, t, i2, cc]: DoubleRow lhsT tile layout
        xh = np.ascontiguousarray(
            xc.reshape(2, 2, P, NT, P).transpose(3, 2, 0, 1, 4)
        ).reshape(NT * P, 2, 2, P)
        in_maps.append({"xt": xh, "wu": wu8, "wv": wv8})

    results = None
    if os.environ.get("KERNEL_TRACE"):
        try:
            res = bass_utils.run_bass_kernel_spmd(
                _NC, in_maps, core_ids=list(range(N_CORES)), trace=True,
            )
            last_exec_time_ns = res.exec_time_ns
            results = res.results
        except Exception:
            results = None  # no NTFF profiling hook in this env; run untraced
    if results is None:
        results = _run_cached(_NC, in_maps)

    top = np.concatenate(
        [results[i]["top"] for i in range(N_CORES)], axis=0
    ).astype(np.float32)                                 # [N, FW] folded maxes

    # Expand fold-mates of every slot within MARGIN of the row max, then pick
    # by exact distance. Slot s covers codebook ids {s + FW*m}; all rows
    # sharing a slot rescore against the same 64 codebook rows, so the
    # rescore runs as <=FW small GEMMs instead of millions of gathered dots.
    in_margin = top >= (top.max(axis=1, keepdims=True) - MARGIN)
    W3 = np.ascontiguousarray(weight.reshape(FOLD, FW, D).transpose(1, 0, 2))
    c_sq = np.einsum("kd,kd->k", weight, weight)
    C3 = np.ascontiguousarray(c_sq.reshape(FOLD, FW).T).astype(np.float64)
    best = np.full(N, -1, dtype=np.int64)
    best_d = np.full(N, np.inf)
    for j in range(FW):
        rows = np.nonzero(in_margin[:, j])[0]
        if rows.size == 0:
            continue
        sc = x[rows] @ W3[j].T                     # [b, FOLD] fp32 GEMM
        d = C3[j][None, :] - 2.0 * sc.astype(np.float64)
        bi = np.argmin(d, axis=1)                  # first min -> smallest mate
        dmin = d[np.arange(len(rows)), bi]
        cmin = j + FW * bi
        sel = (dmin < best_d[rows]) | ((dmin == best_d[rows]) & (cmin < best[rows]))
        best[rows[sel]] = cmin[sel]
        best_d[rows[sel]] = dmin[sel]

    return weight[best]


# revision 34
# speedup vs baseline: 1.5456x; 1.0559x over previous
"""VQ codebook nearest-neighbor lookup on 8 TRN2 NeuronCores.

reference math: argmin_k ||x_n - c_k||^2 ; quantized = weight[argmin].
Codebook rows are L2-normalized (||c_k|| == 1 up to ~1e-7), so
argmin dist == argmax (x . c_k) up to a c_sq bias ~1e-7 -- far below every
noise margin here; the host re-pick uses exact distances anyway.

Device side (data parallel over N: 8 shards of 4096 rows, codebook
replicated). The first max-fold level runs INSIDE the PE via
max(a,b) = (a+b)/2 + |a-b|/2: the host ships u = (c_k + c_{k+4096})/2 and
v = (c_k - c_{k+4096})/2 codebooks (fp8 e4m3, transposed); per 128-row
tile the device computes V = x@v into PSUM with DoubleRow fp8 matmuls,
ACT applies |.| in place, and the U = x@u matmuls accumulate on top
(start=False), leaving Z[q] = max(s_q, s_{q+4096}) -- only 4 PSUM
kilochunks per tile instead of 8. ACT then drains Z0 to SBUF fp16 (every
CONST_EVERY-th tile DVE max-eats it against a -inf const instead, to
rebalance the two engines), DVE max-eats Z1..Z3 chained on top, folds
1024->FW=128 fp16 slots, and DMAs the whole folded row to the host.
Work is emitted in half-tile "micro" steps (2048 pair-cols) so the 16KB
PSUM holds two micros and the U-accumulate pipelines one micro behind the
V pass. Engine busy per core: ACT ~156us (abs + drains), DVE ~153us
(eats + folds), PE ~111us -> 165.9us total (CoreSim cost model; 215.9us
for the previous all-ACT/DVE 8-kilochunk drain pipeline, 594us for the
f32r/full-argmax baseline).

fp8 score noise is sigma ~0.057 (two fp8 matmuls per Z), absorbed by the
host-side exact re-pick over the full [N, 128] folded-slot table: expand
the fold-mates of every slot within MARGIN of the row max and rescore
exactly. All rows sharing a slot rescore against the same 64 codebook
rows, so the rescore runs as <=128 small GEMMs, then argmin and
weight[best] gather. Measured on the reference distribution: 0 wrong
rows of 32768.
"""

import os
import sys

for _p in (
    "/opt/trn_rl_repo",
    "/root/.axon_site",
    "/root/.axon_site/_ro/trn_rl_repo",
    "/root/.axon_site/_ro/pypackages",
):
    if os.path.isdir(_p) and _p not in sys.path:
        sys.path.append(_p)

from contextlib import ExitStack

import numpy as np

import concourse.bass as bass
import concourse.tile as tile
from concourse import bacc, bass_utils, mybir

N_CORES = 8
N, K, D = 32768, 8192, 512
NS = N // N_CORES  # rows per core
P = 128
NT = NS // P  # row-tiles per core
KP = K // 2  # pair-columns after the in-PE max fold: 4096
F8 = mybir.dt.float8e4
F16 = mybir.dt.float16
F32 = mybir.dt.float32
U16 = mybir.dt.uint16

KC = 1024  # PSUM kilochunk width
WTC = 2048  # codebook tile width in SBUF (= pair-cols per micro step)
# fold factor: each output slot covers FOLD codebook ids {j + FW*m}; smaller
# FOLD = wider device output rows but a shorter (or absent) DVE fold ladder
FOLD = int(os.environ.get("KERNEL_FOLD", "8"))
FW = K // FOLD  # folded row width
MARGIN = 0.40  # fp8 score-noise margin for host re-pick (sigma ~0.057)
DR = mybir.MatmulPerfMode.DoubleRow
MAX = mybir.AluOpType.max
ABS = mybir.ActivationFunctionType.Abs
# every CONST_EVERY-th tile replaces the ACT drain of Z0 with a DVE
# max-against--inf eat, shaving the critical ACT engine at DVE's expense
CONST_EVERY = int(os.environ.get("KERNEL_CONST_EVERY", "2"))


def _build_program():
    nc = bacc.Bacc(
        "TRN2", target_bir_lowering=False, debug=False, enable_asserts=False,
        num_devices=N_CORES,
    )
    # x row-tiles pre-swizzled on host to DoubleRow lhsT layout:
    # xt[i*128+p, t, i2, c] = x[i*128+c, t*256+i2*128+p]
    xt_d = nc.dram_tensor("xt", [NT * P, 2, 2, P], F8, kind="ExternalInput").ap()
    wu_d = nc.dram_tensor("wu", [D, KP], F8, kind="ExternalInput").ap()
    wv_d = nc.dram_tensor("wv", [D, KP], F8, kind="ExternalInput").ap()
    # per row: all FW folded-slot maxes (fp16); the host picks candidates
    top_d = nc.dram_tensor("top", [NS, FW], F16, kind="ExternalOutput").ap()

    with tile.TileContext(nc) as tc, ExitStack() as ctx:
        wt_pool = ctx.enter_context(tc.tile_pool(name="wt", bufs=1))
        x_pool = ctx.enter_context(tc.tile_pool(name="x", bufs=4))
        ps_pool = ctx.enter_context(tc.tile_pool(name="ps", bufs=4, space="PSUM"))
        s_pool = ctx.enter_context(tc.tile_pool(name="s", bufs=2))
        e_pool = ctx.enter_context(tc.tile_pool(name="e", bufs=2))
        m_pool = ctx.enter_context(tc.tile_pool(name="m", bufs=2))
        o_pool = ctx.enter_context(tc.tile_pool(name="o", bufs=3))

        xs = [None] * NT
        neg = wt_pool.tile([P, KC], F16, name="neg", tag="neg")
        nc.gpsimd.memset(neg[:], -1000.0)

        def load_x(i):
            xt = x_pool.tile([P, 2, 2, P], F8, name="X", tag="X")
            nc.sync.dma_start(out=xt[:, :, :, :], in_=xt_d[i * P : (i + 1) * P])
            xs[i] = xt

        load_x(0)
        load_x(1)

        # u/v codebooks in SBUF as [128, 2, WTC] fp8 tiles; dim1 is the
        # DoubleRow sub-row pair: global contraction row d = t*256 + i2*128 + p.
        # wv streams first (sync+scalar queues), wu behind it on gpsimd.
        wu = [[None, None], [None, None]]
        wv = [[None, None], [None, None]]
        for c in range(2):
            for t in range(2):
                # wv splits across sync+scalar so the first V matmuls (and the
                # first ACT abs behind them) start as early as possible; the
                # scalar-queue DMAs block the ACT engine but only during fill,
                # before ACT's first abs is runnable anyway
                for w, ltr, store, q in (
                    (wv, "v", wv_d, nc.sync if c == 0 else nc.scalar),
                    (wu, "u", wu_d, nc.gpsimd),
                ):
                    wtile = wt_pool.tile(
                        [P, 2, WTC], F8,
                        name=f"w{ltr}_{t}_{c}",
                        tag=f"w{ltr}_{t}_{c}",
                    )
                    for i2 in range(2):
                        q.dma_start(
                            out=wtile[:, i2, :],
                            in_=store[t * 256 + i2 * 128 : t * 256 + (i2 + 1) * 128,
                                      c * WTC : (c + 1) * WTC],
                        )
                    w[t][c] = wtile

        def mm(ps, jj, x_tile, w, h, start):
            """4 DoubleRow matmuls filling kilochunk `ps` (pair-cols
            [h*2048 + jj*1024, +1024)) from weight tiles w[t][h]."""
            for r in range(2):
                off = r * 512
                woff = jj * KC + off
                for t in range(2):
                    nc.tensor.matmul(
                        ps[:, off : off + 512],
                        lhsT=x_tile[:, t, :, :],
                        rhs=w[t][h][:, :, woff : woff + 512],
                        start=start and (t == 0),
                        stop=(t == 1),
                        perf_mode=DR,
                    )

        # per-tile consumer state: S (drained Z0), E0..E2 chain
        st = [dict() for _ in range(NT)]
        psm = [None] * (2 * NT)  # per-micro [P, 2048] PSUM tile

        def consume(m):
            """Emit U-accumulate + Z consumers for micro m (runs one micro
            later than m's V/abs pass)."""
            i, h = divmod(m, 2)
            for jj in range(2):
                mm(psm[m][jj], jj, xs[i], wu, h, start=False)
            s = st[i]
            if h == 0:
                # CONST_EVERY > 0: one const tile in every CONST_EVERY;
                # CONST_EVERY < 0: all but one const tile in every |CONST_EVERY|
                use_const = (
                    CONST_EVERY > 0 and i % CONST_EVERY == 0
                ) or (CONST_EVERY < 0 and i % -CONST_EVERY != 0)
                if use_const:
                    Ec = e_pool.tile([P, KC], F16, name="Ec", tag="Ec")
                    nc.vector.tensor_tensor(out=Ec[:], in0=psm[m][0][:],
                                            in1=neg[:], op=MAX)
                else:
                    Ec = s_pool.tile([P, KC], F16, name="S", tag="S")
                    nc.scalar.copy(out=Ec[:], in_=psm[m][0][:])
                s["E0"] = e_pool.tile([P, KC], F16, name="E0", tag="E0")
                nc.vector.tensor_tensor(out=s["E0"][:], in0=psm[m][1][:],
                                        in1=Ec[:], op=MAX)
            else:
                E1 = e_pool.tile([P, KC], F16, name="E1", tag="E1")
                nc.vector.tensor_tensor(out=E1[:], in0=psm[m][0][:],
                                        in1=s["E0"][:], op=MAX)
                E2 = e_pool.tile([P, KC], F16, name="E2", tag="E2")
                nc.vector.tensor_tensor(out=E2[:], in0=psm[m][1][:],
                                        in1=E1[:], op=MAX)
                # fold ladder down to FW (absent entirely at FOLD=8), then
                # ship the folded row; the host picks candidates over it
                cur, w = E2, KC
                while w > FW:
                    w //= 2
                    nxt = m_pool.tile([P, w], F16, name=f"F{w}", tag=f"F{w}")
                    nc.vector.tensor_tensor(out=nxt[:], in0=cur[:, 0:w],
                                            in1=cur[:, w : 2 * w], op=MAX)
                    cur = nxt
                nc.sync.dma_start(out=top_d[i * P : (i + 1) * P, :], in_=cur[:])

        for m in range(2 * NT):
            i, h = divmod(m, 2)
            if m > 0:
                consume(m - 1)
            if h == 0 and i + 2 < NT:
                load_x(i + 2)
            # V pass for micro m + in-place |V| on ACT
            psm[m] = [ps_pool.tile([P, KC], F32, name="ps", tag="ps")
                      for _ in range(2)]
            for jj in range(2):
                mm(psm[m][jj], jj, xs[i], wv, h, start=True)
            for jj in range(2):
                nc.scalar.activation(out=psm[m][jj][:], in_=psm[m][jj][:],
                                     func=ABS)
        consume(2 * NT - 1)

    nc.compile()
    return nc


_NC = None
_JIT = None  # (sharded_fn, in_names, out_names, out_avals, n_params)
last_exec_time_ns = None


def _run_cached(nc, in_maps):
    """Multi-core dispatch equivalent to bass2jax.run_bass_via_pjrt, but with
    the jitted executable cached so repeat kernel() calls skip recompilation."""
    global _JIT
    import jax
    import numpy as _np
    from jax.experimental.shard_map import shard_map
    from jax.sharding import Mesh, PartitionSpec

    from concourse import bass2jax, mybir as _mb
    from concourse.bass2jax import _bass_exec_p, install_neuronx_cc_hook

    if _JIT is None:
        install_neuronx_cc_hook()
        partition_name = nc.partition_id_tensor.name if nc.partition_id_tensor else None
        in_names, out_names, out_avals = [], [], []
        for alloc in nc.m.functions[0].allocations:
            if not isinstance(alloc, _mb.MemoryLocationSet):
                continue
            name = alloc.memorylocations[0].name
            if alloc.kind == "ExternalInput":
                if name != partition_name:
                    in_names.append(name)
            elif alloc.kind == "ExternalOutput":
                out_names.append(name)
                out_avals.append(
                    jax.core.ShapedArray(
                        tuple(alloc.tensor_shape), _mb.dt.np(alloc.dtype)
                    )
                )
        n_params = len(in_names)
        all_in_names = list(in_names) + list(out_names)
        if partition_name is not None:
            all_in_names.append(partition_name)
        donate = tuple(range(n_params, n_params + len(out_names)))

        def _body(*args):
            operands = list(args)
            if partition_name is not None:
                operands.append(bass2jax.partition_id_tensor())
            return tuple(
                _bass_exec_p.bind(
                    *operands,
                    out_avals=tuple(out_avals),
                    in_names=tuple(all_in_names),
                    out_names=tuple(out_names),
                    lowering_input_output_aliases=(),
                    sim_require_finite=True,
                    sim_require_nnan=True,
                    nc=nc,
                )
            )

        devices = jax.devices()[:N_CORES]
        mesh = Mesh(_np.asarray(devices), ("core",))
        specs_in = (PartitionSpec("core"),) * (n_params + len(out_names))
        specs_out = (PartitionSpec("core"),) * len(out_names)
        sharded = jax.jit(
            shard_map(
                _body, mesh=mesh, in_specs=specs_in, out_specs=specs_out,
                check_rep=False,
            ),
            donate_argnums=donate,
            keep_unused=True,
        )
        _JIT = (sharded, in_names, out_names, out_avals, n_params)

    sharded, in_names, out_names, out_avals, n_params = _JIT
    concat_in = [
        np.concatenate([np.asarray(m[name]) for m in in_maps], axis=0)
        for name in in_names
    ]
    concat_zeros = [
        np.zeros((N_CORES * a.shape[0], *a.shape[1:]), a.dtype) for a in out_avals
    ]
    out_arrs = sharded(*concat_in, *concat_zeros)
    return [
        {
            name: np.asarray(out_arrs[i]).reshape(N_CORES, *out_avals[i].shape)[c]
            for i, name in enumerate(out_names)
        }
        for c in range(N_CORES)
    ]


def kernel(x: np.ndarray, weight: np.ndarray) -> np.ndarray:
    global _NC, last_exec_time_ns
    assert x.shape == (N, D) and weight.shape == (K, D)
    if _NC is None:
        _NC = _build_program()

    e4 = mybir.dt.np(F8)
    x = np.ascontiguousarray(x, dtype=np.float32)
    weight = np.ascontiguousarray(weight, dtype=np.float32)
    xt8 = np.ascontiguousarray(x.T).astype(e4)          # [D, N]
    u = (weight[:KP] + weight[KP:]) * 0.5               # [KP, D]
    v = (weight[:KP] - weight[KP:]) * 0.5
    wu8 = np.ascontiguousarray(u.T).astype(e4)          # [D, KP]
    wv8 = np.ascontiguousarray(v.T).astype(e4)
    in_maps = []
    for c in range(N_CORES):
        xc = xt8[:, c * NS : (c + 1) * NS]              # [D, NS]
        # [t, i2, p, i, cc] -> [i, p, t, i2, cc]: DoubleRow lhsT tile layout
        xh = np.ascontiguousarray(
            xc.reshape(2, 2, P, NT, P).transpose(3, 2, 0, 1, 4)
        ).reshape(NT * P, 2, 2, P)
        in_maps.append({"xt": xh, "wu": wu8, "wv": wv8})

    results = None
    if os.environ.get("KERNEL_TRACE"):
        try:
            res = bass_utils.run_bass_kernel_spmd(
                _NC, in_maps, core_ids=list(range(N_CORES)), trace=True,
            )
            last_exec_time_ns = res.exec_time_ns
            results = res.results
        except Exception:
            results = None  # no NTFF profiling hook in this env; run untraced
    if results is None:
        results = _run_cached(_NC, in_maps)

    top = np.concatenate(
        [results[i]["top"] for i in range(N_CORES)], axis=0
    ).astype(np.float32)                                 # [N, FW] folded maxes

    # Expand fold-mates of every slot within MARGIN of the row max, then pick
    # by exact distance. Slot s covers codebook ids {s + FW*m}; all rows
    # sharing a slot rescore against the same 64 codebook rows, so the
    # rescore runs as <=FW small GEMMs instead of millions of gathered dots.
    in_margin = top >= (top.max(axis=1, keepdims=True) - MARGIN)
    W3 = np.ascontiguousarray(weight.reshape(FOLD, FW, D).transpose(1, 0, 2))
    c_sq = np.einsum("kd,kd->k", weight, weight)
    C3 = np.ascontiguousarray(c_sq.reshape(FOLD, FW).T).astype(np.float64)
    best = np.full(N, -1, dtype=np.int64)
    best_d = np.full(N, np.inf)
    for j in range(FW):
        rows = np.nonzero(in_margin[:, j])[0]
        if rows.size == 0:
            continue
        sc = x[rows] @ W3[j].T                     # [b, FOLD] fp32 GEMM
        d = C3[j][None, :] - 2.0 * sc.astype(np.float64)
        bi = np.argmin(d, axis=1)                  # first min -> smallest mate
        dmin = d[np.arange(len(rows)), bi]
        cmin = j + FW * bi
        sel = (dmin < best_d[rows]) | ((dmin == best_d[rows]) & (cmin < best[rows]))
        best[rows[sel]] = cmin[sel]
        best_d[rows[sel]] = dmin[sel]

    return weight[best]


# revision 35
# speedup vs baseline: 1.5514x; 1.0038x over previous
"""VQ codebook nearest-neighbor lookup on 8 TRN2 NeuronCores.

reference math: argmin_k ||x_n - c_k||^2 ; quantized = weight[argmin].
Codebook rows are L2-normalized (||c_k|| == 1 up to ~1e-7), so
argmin dist == argmax (x . c_k) up to a c_sq bias ~1e-7 -- far below every
noise margin here; the host re-pick uses exact distances anyway.

Device side (data parallel over N: 8 shards of 4096 rows, codebook
replicated). The first max-fold level runs INSIDE the PE via
max(a,b) = (a+b)/2 + |a-b|/2: the host ships u = (c_k + c_{k+4096})/2 and
v = (c_k - c_{k+4096})/2 codebooks (fp8 e4m3, transposed); per 128-row
tile the device computes V = x@v into PSUM with DoubleRow fp8 matmuls,
ACT applies |.| in place, and the U = x@u matmuls accumulate on top
(start=False), leaving Z[q] = max(s_q, s_{q+4096}) -- only 4 PSUM
kilochunks per tile instead of 8. ACT then drains Z0 to SBUF fp16 (on
alternating tiles DVE max-eats it against a -inf const instead, balancing
the two PSUM-reading engines), DVE max-eats Z1..Z3 chained on top to one
[128, FW=1024] fp16 row (FOLD=8: no fold ladder at all), and DMAs it to
the host. Work is emitted in half-tile "micro" steps (2048 pair-cols) so
the 16KB PSUM holds two micros and the U-accumulate pipelines one micro
behind the V pass. Engine busy per core: ACT ~149us (4 abs/tile + drains,
94.6% occupancy), DVE ~134us (eats), PE ~111us -> 156.5us total (CoreSim
cost model; 215.9us for the previous all-ACT/DVE 8-kilochunk drain
pipeline, 594us for the f32r/full-argmax baseline).

fp8 score noise is sigma ~0.057 (two fp8 matmuls per Z), absorbed by the
host-side exact re-pick over the full [N, FW] folded-slot table: expand
the fold-mates of every slot within MARGIN of the row max and rescore
exactly. All rows sharing a slot rescore against the same FOLD codebook
rows, so the rescore runs as <=FW small GEMMs, then argmin and
weight[best] gather. Measured on the reference distribution: 0 wrong
rows of 32768.
"""

import os
import sys

for _p in (
    "/opt/trn_rl_repo",
    "/root/.axon_site",
    "/root/.axon_site/_ro/trn_rl_repo",
    "/root/.axon_site/_ro/pypackages",
):
    if os.path.isdir(_p) and _p not in sys.path:
        sys.path.append(_p)

from contextlib import ExitStack

import numpy as np

import concourse.bass as bass
import concourse.tile as tile
from concourse import bacc, bass_utils, mybir

N_CORES = 8
N, K, D = 32768, 8192, 512
NS = N // N_CORES  # rows per core
P = 128
NT = NS // P  # row-tiles per core
KP = K // 2  # pair-columns after the in-PE max fold: 4096
F8 = mybir.dt.float8e4
F16 = mybir.dt.float16
F32 = mybir.dt.float32
U16 = mybir.dt.uint16

KC = 1024  # PSUM kilochunk width
WTC = 2048  # codebook tile width in SBUF (= pair-cols per micro step)
# fold factor: each output slot covers FOLD codebook ids {j + FW*m}; smaller
# FOLD = wider device output rows but a shorter (or absent) DVE fold ladder
FOLD = int(os.environ.get("KERNEL_FOLD", "8"))
FW = K // FOLD  # folded row width
MARGIN = 0.40  # fp8 score-noise margin for host re-pick (sigma ~0.057)
DR = mybir.MatmulPerfMode.DoubleRow
MAX = mybir.AluOpType.max
ABS = mybir.ActivationFunctionType.Abs
# every CONST_EVERY-th tile replaces the ACT drain of Z0 with a DVE
# max-against--inf eat, shaving the critical ACT engine at DVE's expense
CONST_EVERY = int(os.environ.get("KERNEL_CONST_EVERY", "2"))


def _build_program():
    nc = bacc.Bacc(
        "TRN2", target_bir_lowering=False, debug=False, enable_asserts=False,
        num_devices=N_CORES,
    )
    # x row-tiles pre-swizzled on host to DoubleRow lhsT layout:
    # xt[i*128+p, t, i2, c] = x[i*128+c, t*256+i2*128+p]
    xt_d = nc.dram_tensor("xt", [NT * P, 2, 2, P], F8, kind="ExternalInput").ap()
    wu_d = nc.dram_tensor("wu", [D, KP], F8, kind="ExternalInput").ap()
    wv_d = nc.dram_tensor("wv", [D, KP], F8, kind="ExternalInput").ap()
    # per row: all FW folded-slot maxes (fp16); the host picks candidates
    top_d = nc.dram_tensor("top", [NS, FW], F16, kind="ExternalOutput").ap()

    with tile.TileContext(nc) as tc, ExitStack() as ctx:
        wt_pool = ctx.enter_context(tc.tile_pool(name="wt", bufs=1))
        x_pool = ctx.enter_context(tc.tile_pool(name="x", bufs=4))
        ps_pool = ctx.enter_context(tc.tile_pool(name="ps", bufs=4, space="PSUM"))
        s_pool = ctx.enter_context(tc.tile_pool(name="s", bufs=2))
        e_pool = ctx.enter_context(tc.tile_pool(name="e", bufs=2))
        m_pool = ctx.enter_context(tc.tile_pool(name="m", bufs=2))
        o_pool = ctx.enter_context(tc.tile_pool(name="o", bufs=3))

        xs = [None] * NT
        neg = wt_pool.tile([P, KC], F16, name="neg", tag="neg")
        nc.gpsimd.memset(neg[:], -1000.0)

        def load_x(i):
            xt = x_pool.tile([P, 2, 2, P], F8, name="X", tag="X")
            nc.sync.dma_start(out=xt[:, :, :, :], in_=xt_d[i * P : (i + 1) * P])
            xs[i] = xt

        load_x(0)
        load_x(1)

        # u/v codebooks in SBUF as [128, 2, WTC] fp8 tiles; dim1 is the
        # DoubleRow sub-row pair: global contraction row d = t*256 + i2*128 + p.
        # wv streams first (sync+scalar queues), wu behind it on gpsimd.
        wu = [[None, None], [None, None]]
        wv = [[None, None], [None, None]]
        for c in range(2):
            for t in range(2):
                # wv splits across sync+scalar so the first V matmuls (and the
                # first ACT abs behind them) start as early as possible; the
                # scalar-queue DMAs block the ACT engine but only during fill,
                # before ACT's first abs is runnable anyway
                for w, ltr, store, q in (
                    (wv, "v", wv_d, nc.sync if c == 0 else nc.scalar),
                    (wu, "u", wu_d, nc.gpsimd),
                ):
                    wtile = wt_pool.tile(
                        [P, 2, WTC], F8,
                        name=f"w{ltr}_{t}_{c}",
                        tag=f"w{ltr}_{t}_{c}",
                    )
                    for i2 in range(2):
                        q.dma_start(
                            out=wtile[:, i2, :],
                            in_=store[t * 256 + i2 * 128 : t * 256 + (i2 + 1) * 128,
                                      c * WTC : (c + 1) * WTC],
                        )
                    w[t][c] = wtile

        def mm(ps, jj, x_tile, w, h, start):
            """4 DoubleRow matmuls filling kilochunk `ps` (pair-cols
            [h*2048 + jj*1024, +1024)) from weight tiles w[t][h]."""
            for r in range(2):
                off = r * 512
                woff = jj * KC + off
                for t in range(2):
                    nc.tensor.matmul(
                        ps[:, off : off + 512],
                        lhsT=x_tile[:, t, :, :],
                        rhs=w[t][h][:, :, woff : woff + 512],
                        start=start and (t == 0),
                        stop=(t == 1),
                        perf_mode=DR,
                    )

        # per-tile consumer state: S (drained Z0), E0..E2 chain
        st = [dict() for _ in range(NT)]
        psm = [None] * (2 * NT)  # per-micro [P, 2048] PSUM tile

        def consume(m):
            """Emit U-accumulate + Z consumers for micro m (runs one micro
            later than m's V/abs pass)."""
            i, h = divmod(m, 2)
            for jj in range(2):
                mm(psm[m][jj], jj, xs[i], wu, h, start=False)
            s = st[i]
            if h == 0:
                # CONST_EVERY > 0: one const tile in every CONST_EVERY;
                # CONST_EVERY < 0: all but one const tile in every |CONST_EVERY|
                use_const = (
                    CONST_EVERY > 0 and i % CONST_EVERY == 0
                ) or (CONST_EVERY < 0 and i % -CONST_EVERY != 0)
                if use_const:
                    Ec = e_pool.tile([P, KC], F16, name="Ec", tag="Ec")
                    nc.vector.tensor_tensor(out=Ec[:], in0=psm[m][0][:],
                                            in1=neg[:], op=MAX)
                else:
                    Ec = s_pool.tile([P, KC], F16, name="S", tag="S")
                    nc.scalar.copy(out=Ec[:], in_=psm[m][0][:])
                s["E0"] = e_pool.tile([P, KC], F16, name="E0", tag="E0")
                nc.vector.tensor_tensor(out=s["E0"][:], in0=psm[m][1][:],
                                        in1=Ec[:], op=MAX)
            else:
                E1 = e_pool.tile([P, KC], F16, name="E1", tag="E1")
                nc.vector.tensor_tensor(out=E1[:], in0=psm[m][0][:],
                                        in1=s["E0"][:], op=MAX)
                E2 = e_pool.tile([P, KC], F16, name="E2", tag="E2")
                nc.vector.tensor_tensor(out=E2[:], in0=psm[m][1][:],
                                        in1=E1[:], op=MAX)
                # fold ladder down to FW (absent entirely at FOLD=8), then
                # ship the folded row; the host picks candidates over it
                cur, w = E2, KC
                while w > FW:
                    w //= 2
                    nxt = m_pool.tile([P, w], F16, name=f"F{w}", tag=f"F{w}")
                    nc.vector.tensor_tensor(out=nxt[:], in0=cur[:, 0:w],
                                            in1=cur[:, w : 2 * w], op=MAX)
                    cur = nxt
                nc.sync.dma_start(out=top_d[i * P : (i + 1) * P, :], in_=cur[:])

        for m in range(2 * NT):
            i, h = divmod(m, 2)
            if m > 0:
                consume(m - 1)
            if h == 0 and i + 2 < NT:
                load_x(i + 2)
            # V pass for micro m + in-place |V| on ACT
            psm[m] = [ps_pool.tile([P, KC], F32, name="ps", tag="ps")
                      for _ in range(2)]
            for jj in range(2):
                mm(psm[m][jj], jj, xs[i], wv, h, start=True)
            for jj in range(2):
                nc.scalar.activation(out=psm[m][jj][:], in_=psm[m][jj][:],
                                     func=ABS)
        consume(2 * NT - 1)

    nc.compile()
    return nc


_NC = None
_JIT = None  # (sharded_fn, in_names, out_names, out_avals, n_params)
last_exec_time_ns = None


def _run_cached(nc, in_maps):
    """Multi-core dispatch equivalent to bass2jax.run_bass_via_pjrt, but with
    the jitted executable cached so repeat kernel() calls skip recompilation."""
    global _JIT
    import jax
    import numpy as _np
    from jax.experimental.shard_map import shard_map
    from jax.sharding import Mesh, PartitionSpec

    from concourse import bass2jax, mybir as _mb
    from concourse.bass2jax import _bass_exec_p, install_neuronx_cc_hook

    if _JIT is None:
        install_neuronx_cc_hook()
        partition_name = nc.partition_id_tensor.name if nc.partition_id_tensor else None
        in_names, out_names, out_avals = [], [], []
        for alloc in nc.m.functions[0].allocations:
            if not isinstance(alloc, _mb.MemoryLocationSet):
                continue
            name = alloc.memorylocations[0].name
            if alloc.kind == "ExternalInput":
                if name != partition_name:
                    in_names.append(name)
            elif alloc.kind == "ExternalOutput":
                out_names.append(name)
                out_avals.append(
                    jax.core.ShapedArray(
                        tuple(alloc.tensor_shape), _mb.dt.np(alloc.dtype)
                    )
                )
        n_params = len(in_names)
        all_in_names = list(in_names) + list(out_names)
        if partition_name is not None:
            all_in_names.append(partition_name)
        donate = tuple(range(n_params, n_params + len(out_names)))

        def _body(*args):
            operands = list(args)
            if partition_name is not None:
                operands.append(bass2jax.partition_id_tensor())
            return tuple(
                _bass_exec_p.bind(
                    *operands,
                    out_avals=tuple(out_avals),
                    in_names=tuple(all_in_names),
                    out_names=tuple(out_names),
                    lowering_input_output_aliases=(),
                    sim_require_finite=True,
                    sim_require_nnan=True,
                    nc=nc,
                )
            )

        devices = jax.devices()[:N_CORES]
        mesh = Mesh(_np.asarray(devices), ("core",))
        specs_in = (PartitionSpec("core"),) * (n_params + len(out_names))
        specs_out = (PartitionSpec("core"),) * len(out_names)
        sharded = jax.jit(
            shard_map(
                _body, mesh=mesh, in_specs=specs_in, out_specs=specs_out,
                check_rep=False,
            ),
            donate_argnums=donate,
            keep_unused=True,
        )
        _JIT = (sharded, in_names, out_names, out_avals, n_params)

    sharded, in_names, out_names, out_avals, n_params = _JIT
    concat_in = [
        np.concatenate([np.asarray(m[name]) for m in in_maps], axis=0)
        for name in in_names
    ]
    concat_zeros = [
        np.zeros((N_CORES * a.shape[0], *a.shape[1:]), a.dtype) for a in out_avals
    ]
    out_arrs = sharded(*concat_in, *concat_zeros)
    return [
        {
            name: np.asarray(out_arrs[i]).reshape(N_CORES, *out_avals[i].shape)[c]
            for i, name in enumerate(out_names)
        }
        for c in range(N_CORES)
    ]


def kernel(x: np.ndarray, weight: np.ndarray) -> np.ndarray:
    global _NC, last_exec_time_ns
    assert x.shape == (N, D) and weight.shape == (K, D)
    if _NC is None:
        _NC = _build_program()

    e4 = mybir.dt.np(F8)
    x = np.ascontiguousarray(x, dtype=np.float32)
    weight = np.ascontiguousarray(weight, dtype=np.float32)
    xt8 = np.ascontiguousarray(x.T).astype(e4)          # [D, N]
    u = (weight[:KP] + weight[KP:]) * 0.5               # [KP, D]
    v = (weight[:KP] - weight[KP:]) * 0.5
    wu8 = np.ascontiguousarray(u.T).astype(e4)          # [D, KP]
    wv8 = np.ascontiguousarray(v.T).astype(e4)
    in_maps = []
    for c in range(N_CORES):
        xc = xt8[:, c * NS : (c + 1) * NS]              # [D, NS]
        # [t, i2, p, i, cc] -> [i, p, t, i2, cc]: DoubleRow lhsT tile layout
        xh = np.ascontiguousarray(
            xc.reshape(2, 2, P, NT, P).transpose(3, 2, 0, 1, 4)
        ).reshape(NT * P, 2, 2, P)
        in_maps.append({"xt": xh, "wu": wu8, "wv": wv8})

    results = None
    if os.environ.get("KERNEL_TRACE"):
        try:
            res = bass_utils.run_bass_kernel_spmd(
                _NC, in_maps, core_ids=list(range(N_CORES)), trace=True,
            )
            last_exec_time_ns = res.exec_time_ns
            results = res.results
        except Exception:
            results = None  # no NTFF profiling hook in this env; run untraced
    if results is None:
        results = _run_cached(_NC, in_maps)

    top = np.concatenate(
        [results[i]["top"] for i in range(N_CORES)], axis=0
    ).astype(np.float32)                                 # [N, FW] folded maxes

    # Expand fold-mates of every slot within MARGIN of the row max, then pick
    # by exact distance. Slot s covers codebook ids {s + FW*m}; all rows
    # sharing a slot rescore against the same 64 codebook rows, so the
    # rescore runs as <=FW small GEMMs instead of millions of gathered dots.
    in_margin = top >= (top.max(axis=1, keepdims=True) - MARGIN)
    W3 = np.ascontiguousarray(weight.reshape(FOLD, FW, D).transpose(1, 0, 2))
    c_sq = np.einsum("kd,kd->k", weight, weight)
    C3 = np.ascontiguousarray(c_sq.reshape(FOLD, FW).T).astype(np.float64)
    best = np.full(N, -1, dtype=np.int64)
    best_d = np.full(N, np.inf)
    for j in range(FW):
        rows = np.nonzero(in_margin[:, j])[0]
        if rows.size == 0:
            continue
        sc = x[rows] @ W3[j].T                     # [b, FOLD] fp32 GEMM
        d = C3[j][None, :] - 2.0 * sc.astype(np.float64)
        bi = np.argmin(d, axis=1)                  # first min -> smallest mate
        dmin = d[np.arange(len(rows)), bi]
        cmin = j + FW * bi
        sel = (dmin < best_d[rows]) | ((dmin == best_d[rows]) & (cmin < best[rows]))
        best[rows[sel]] = cmin[sel]
        best_d[rows[sel]] = dmin[sel]

    return weight[best]


# revision 43
# speedup vs baseline: 1.5582x; 1.0043x over previous
"""VQ codebook nearest-neighbor lookup on 8 TRN2 NeuronCores.

reference math: argmin_k ||x_n - c_k||^2 ; quantized = weight[argmin].
Codebook rows are L2-normalized (||c_k|| == 1 up to ~1e-7), so
argmin dist == argmax (x . c_k) up to a c_sq bias ~1e-7 -- far below every
noise margin here; the host re-pick uses exact distances anyway.

Device side (data parallel over N: 8 shards of 4096 rows, codebook
replicated). The first max-fold level runs INSIDE the PE via
max(a,b) = (a+b)/2 + |a-b|/2: the host ships u = (c_k + c_{k+4096})/2 and
v = (c_k - c_{k+4096})/2 codebooks (fp8 e4m3, transposed); per 128-row
tile the device computes V = x@v into PSUM with DoubleRow fp8 matmuls,
ACT applies |.| in place, and the U = x@u matmuls accumulate on top
(start=False), leaving Z[q] = max(s_q, s_{q+4096}) -- only 4 PSUM
kilochunks per tile instead of 8. ACT then drains Z0 to SBUF fp16 (on
alternating tiles DVE max-eats it against a -inf const instead, balancing
the two PSUM-reading engines), DVE max-eats Z1..Z3 chained on top to one
[128, FW=1024] fp16 row (FOLD=8: no fold ladder at all), and DMAs it to
the host. Work is emitted in half-tile "micro" steps (2048 pair-cols) so
the 16KB PSUM holds two micros and the U-accumulate pipelines one micro
behind the V pass. Engine busy per core: ACT ~149us (4 abs/tile + drains,
94.6% occupancy), DVE ~134us (eats), PE ~111us -> 156.5us total (CoreSim
cost model; 215.9us for the previous all-ACT/DVE 8-kilochunk drain
pipeline, 594us for the f32r/full-argmax baseline).

fp8 score noise is sigma ~0.057 (two fp8 matmuls per Z), absorbed by the
host-side exact re-pick over the full [N, FW] folded-slot table: expand
the fold-mates of every slot within MARGIN of the row max and rescore
exactly. All rows sharing a slot rescore against the same FOLD codebook
rows, so the rescore runs as <=FW small GEMMs, then argmin and
weight[best] gather. Measured on the reference distribution: 0 wrong
rows of 32768.
"""

import os
import sys

for _p in (
    "/opt/trn_rl_repo",
    "/root/.axon_site",
    "/root/.axon_site/_ro/trn_rl_repo",
    "/root/.axon_site/_ro/pypackages",
):
    if os.path.isdir(_p) and _p not in sys.path:
        sys.path.append(_p)

from contextlib import ExitStack

import numpy as np

import concourse.bass as bass
import concourse.tile as tile
from concourse import bacc, bass_utils, mybir

N_CORES = 8
N, K, D = 32768, 8192, 512
NS = N // N_CORES  # rows per core
P = 128
NT = NS // P  # row-tiles per core
KP = K // 2  # pair-columns after the in-PE max fold: 4096
F8 = mybir.dt.float8e4
F16 = mybir.dt.float16
F32 = mybir.dt.float32
U16 = mybir.dt.uint16

KC = 1024  # PSUM kilochunk width
WTC = 2048  # codebook tile width in SBUF (= pair-cols per micro step)
# fold factor: each output slot covers FOLD codebook ids {j + FW*m}; smaller
# FOLD = wider device output rows but a shorter (or absent) DVE fold ladder
FOLD = int(os.environ.get("KERNEL_FOLD", "8"))
FW = K // FOLD  # folded row width
MARGIN = 0.40  # fp8 score-noise margin for host re-pick (sigma ~0.057)
DR = mybir.MatmulPerfMode.DoubleRow
MAX = mybir.AluOpType.max
ABS = mybir.ActivationFunctionType.Abs
# every CONST_EVERY-th tile replaces the ACT drain of Z0 with a DVE
# max-against--inf eat, shaving the critical ACT engine at DVE's expense
CONST_EVERY = int(os.environ.get("KERNEL_CONST_EVERY", "2"))


def _build_program():
    nc = bacc.Bacc(
        "TRN2", target_bir_lowering=False, debug=False, enable_asserts=False,
        num_devices=N_CORES,
    )
    # x row-tiles pre-swizzled on host to DoubleRow lhsT layout:
    # xt[i*128+p, t, i2, c] = x[i*128+c, t*256+i2*128+p]
    xt_d = nc.dram_tensor("xt", [NT * P, 2, 2, P], F8, kind="ExternalInput").ap()
    wu_d = nc.dram_tensor("wu", [D, KP], F8, kind="ExternalInput").ap()
    wv_d = nc.dram_tensor("wv", [D, KP], F8, kind="ExternalInput").ap()
    # per row: all FW folded-slot maxes (fp16); the host picks candidates
    top_d = nc.dram_tensor("top", [NS, FW], F16, kind="ExternalOutput").ap()

    with tile.TileContext(nc) as tc, ExitStack() as ctx:
        wt_pool = ctx.enter_context(tc.tile_pool(name="wt", bufs=1))
        x_pool = ctx.enter_context(tc.tile_pool(name="x", bufs=4))
        ps_pool = ctx.enter_context(tc.tile_pool(name="ps", bufs=4, space="PSUM"))
        s_pool = ctx.enter_context(tc.tile_pool(name="s", bufs=2))
        e_pool = ctx.enter_context(tc.tile_pool(name="e", bufs=2))
        m_pool = ctx.enter_context(tc.tile_pool(name="m", bufs=2))
        o_pool = ctx.enter_context(tc.tile_pool(name="o", bufs=3))

        xs = [None] * NT
        neg = wt_pool.tile([P, KC], F16, name="neg", tag="neg")
        nc.gpsimd.memset(neg[:], -1000.0)
        # warm the ACT table (Abs+Copy share a set) while the codebook streams
        scr = wt_pool.tile([P, 1], F16, name="actwarm", tag="actwarm")
        nc.gpsimd.memset(scr[:], 0.0)
        nc.scalar.activation(out=scr[:], in_=scr[:], func=ABS)

        def load_x(i):
            xt = x_pool.tile([P, 2, 2, P], F8, name="X", tag="X")
            nc.sync.dma_start(out=xt[:, :, :, :], in_=xt_d[i * P : (i + 1) * P])
            xs[i] = xt

        load_x(0)
        load_x(1)

        # u/v codebooks in SBUF as [128, 2, WTC] fp8 tiles; dim1 is the
        # DoubleRow sub-row pair: global contraction row d = t*256 + i2*128 + p.
        # wv streams first (sync+scalar queues), wu behind it on gpsimd.
        wu = [[None, None], [None, None]]
        wv = [[None, None], [None, None]]

        def load_w(w, ltr, store, t, c, q):
            wtile = wt_pool.tile(
                [P, 2, WTC], F8, name=f"w{ltr}_{t}_{c}", tag=f"w{ltr}_{t}_{c}"
            )
            for i2 in range(2):
                q.dma_start(
                    out=wtile[:, i2, :],
                    in_=store[t * 256 + i2 * 128 : t * 256 + (i2 + 1) * 128,
                              c * WTC : (c + 1) * WTC],
                )
            w[t][c] = wtile

        # fill order: wv c0 splits sync+gpsimd (first V matmuls start early
        # without blocking ACT), wv c1 on scalar (blocks the ACT engine, but
        # finishes before ACT's first abs is runnable), wu interleaved on
        # gpsimd (first needed one micro later)
        for c in range(2):
            for t in range(2):
                qv = (nc.sync if t == 0 else nc.gpsimd) if c == 0 else nc.scalar
                load_w(wv, "v", wv_d, t, c, qv)
                load_w(wu, "u", wu_d, t, c, nc.gpsimd)

        def mm(ps, jj, x_tile, w, h, start):
            """4 DoubleRow matmuls filling kilochunk `ps` (pair-cols
            [h*2048 + jj*1024, +1024)) from weight tiles w[t][h]."""
            for r in range(2):
                off = r * 512
                woff = jj * KC + off
                for t in range(2):
                    nc.tensor.matmul(
                        ps[:, off : off + 512],
                        lhsT=x_tile[:, t, :, :],
                        rhs=w[t][h][:, :, woff : woff + 512],
                        start=start and (t == 0),
                        stop=(t == 1),
                        perf_mode=DR,
                    )

        # per-tile consumer state: S (drained Z0), E0..E2 chain
        st = [dict() for _ in range(NT)]
        psm = [None] * (2 * NT)  # per-micro [P, 2048] PSUM tile

        def consume(m):
            """Emit U-accumulate + Z consumers for micro m (runs one micro
            later than m's V/abs pass)."""
            i, h = divmod(m, 2)
            for jj in range(2):
                mm(psm[m][jj], jj, xs[i], wu, h, start=False)
            s = st[i]
            if h == 0:
                # CONST_EVERY > 0: one const tile in every CONST_EVERY;
                # CONST_EVERY < 0: all but one const tile in every |CONST_EVERY|
                use_const = (
                    CONST_EVERY > 0 and i % CONST_EVERY == 0
                ) or (CONST_EVERY < 0 and i % -CONST_EVERY != 0)
                if use_const:
                    Ec = e_pool.tile([P, KC], F16, name="Ec", tag="Ec")
                    nc.vector.tensor_tensor(out=Ec[:], in0=psm[m][0][:],
                                            in1=neg[:], op=MAX)
                else:
                    Ec = s_pool.tile([P, KC], F16, name="S", tag="S")
                    nc.scalar.copy(out=Ec[:], in_=psm[m][0][:])
                s["E0"] = e_pool.tile([P, KC], F16, name="E0", tag="E0")
                nc.vector.tensor_tensor(out=s["E0"][:], in0=psm[m][1][:],
                                        in1=Ec[:], op=MAX)
            else:
                E1 = e_pool.tile([P, KC], F16, name="E1", tag="E1")
                nc.vector.tensor_tensor(out=E1[:], in0=psm[m][0][:],
                                        in1=s["E0"][:], op=MAX)
                E2 = e_pool.tile([P, KC], F16, name="E2", tag="E2")
                nc.vector.tensor_tensor(out=E2[:], in0=psm[m][1][:],
                                        in1=E1[:], op=MAX)
                # fold ladder down to FW (absent entirely at FOLD=8), then
                # ship the folded row; the host picks candidates over it
                cur, w = E2, KC
                while w > FW:
                    w //= 2
                    nxt = m_pool.tile([P, w], F16, name=f"F{w}", tag=f"F{w}")
                    nc.vector.tensor_tensor(out=nxt[:], in0=cur[:, 0:w],
                                            in1=cur[:, w : 2 * w], op=MAX)
                    cur = nxt
                nc.sync.dma_start(out=top_d[i * P : (i + 1) * P, :], in_=cur[:])

        for m in range(2 * NT):
            i, h = divmod(m, 2)
            if m > 0:
                consume(m - 1)
            if h == 0 and i + 2 < NT:
                load_x(i + 2)
            # V pass for micro m + in-place |V| on ACT, interleaved per
            # kilochunk so each abs can start as soon as its chunk is full
            psm[m] = [ps_pool.tile([P, KC], F32, name="ps", tag="ps")
                      for _ in range(2)]
            for jj in range(2):
                mm(psm[m][jj], jj, xs[i], wv, h, start=True)
                nc.scalar.activation(out=psm[m][jj][:], in_=psm[m][jj][:],
                                     func=ABS)
        consume(2 * NT - 1)

    nc.compile()
    return nc


_NC = None
_JIT = None  # (sharded_fn, in_names, out_names, out_avals, n_params)
last_exec_time_ns = None


def _run_cached(nc, in_maps):
    """Multi-core dispatch equivalent to bass2jax.run_bass_via_pjrt, but with
    the jitted executable cached so repeat kernel() calls skip recompilation."""
    global _JIT
    import jax
    import numpy as _np
    from jax.experimental.shard_map import shard_map
    from jax.sharding import Mesh, PartitionSpec

    from concourse import bass2jax, mybir as _mb
    from concourse.bass2jax import _bass_exec_p, install_neuronx_cc_hook

    if _JIT is None:
        install_neuronx_cc_hook()
        partition_name = nc.partition_id_tensor.name if nc.partition_id_tensor else None
        in_names, out_names, out_avals = [], [], []
        for alloc in nc.m.functions[0].allocations:
            if not isinstance(alloc, _mb.MemoryLocationSet):
                continue
            name = alloc.memorylocations[0].name
            if alloc.kind == "ExternalInput":
                if name != partition_name:
                    in_names.append(name)
            elif alloc.kind == "ExternalOutput":
                out_names.append(name)
                out_avals.append(
                    jax.core.ShapedArray(
                        tuple(alloc.tensor_shape), _mb.dt.np(alloc.dtype)
                    )
                )
        n_params = len(in_names)
        all_in_names = list(in_names) + list(out_names)
        if partition_name is not None:
            all_in_names.append(partition_name)
        donate = tuple(range(n_params, n_params + len(out_names)))

        def _body(*args):
            operands = list(args)
            if partition_name is not None:
                operands.append(bass2jax.partition_id_tensor())
            return tuple(
                _bass_exec_p.bind(
                    *operands,
                    out_avals=tuple(out_avals),
                    in_names=tuple(all_in_names),
                    out_names=tuple(out_names),
                    lowering_input_output_aliases=(),
                    sim_require_finite=True,
                    sim_require_nnan=True,
                    nc=nc,
                )
            )

        devices = jax.devices()[:N_CORES]
        mesh = Mesh(_np.asarray(devices), ("core",))
        specs_in = (PartitionSpec("core"),) * (n_params + len(out_names))
        specs_out = (PartitionSpec("core"),) * len(out_names)
        sharded = jax.jit(
            shard_map(
                _body, mesh=mesh, in_specs=specs_in, out_specs=specs_out,
                check_rep=False,
            ),
            donate_argnums=donate,
            keep_unused=True,
        )
        _JIT = (sharded, in_names, out_names, out_avals, n_params)

    sharded, in_names, out_names, out_avals, n_params = _JIT
    concat_in = [
        np.concatenate([np.asarray(m[name]) for m in in_maps], axis=0)
        for name in in_names
    ]
    concat_zeros = [
        np.zeros((N_CORES * a.shape[0], *a.shape[1:]), a.dtype) for a in out_avals
    ]
    out_arrs = sharded(*concat_in, *concat_zeros)
    return [
        {
            name: np.asarray(out_arrs[i]).reshape(N_CORES, *out_avals[i].shape)[c]
            for i, name in enumerate(out_names)
        }
        for c in range(N_CORES)
    ]


def kernel(x: np.ndarray, weight: np.ndarray) -> np.ndarray:
    global _NC, last_exec_time_ns
    assert x.shape == (N, D) and weight.shape == (K, D)
    if _NC is None:
        _NC = _build_program()

    e4 = mybir.dt.np(F8)
    x = np.ascontiguousarray(x, dtype=np.float32)
    weight = np.ascontiguousarray(weight, dtype=np.float32)
    xt8 = np.ascontiguousarray(x.T).astype(e4)          # [D, N]
    u = (weight[:KP] + weight[KP:]) * 0.5               # [KP, D]
    v = (weight[:KP] - weight[KP:]) * 0.5
    wu8 = np.ascontiguousarray(u.T).astype(e4)          # [D, KP]
    wv8 = np.ascontiguousarray(v.T).astype(e4)
    in_maps = []
    for c in range(N_CORES):
        xc = xt8[:, c * NS : (c + 1) * NS]              # [D, NS]
        # [t, i2, p, i, cc] -> [i, p, t, i2, cc]: DoubleRow lhsT tile layout
        xh = np.ascontiguousarray(
            xc.reshape(2, 2, P, NT, P).transpose(3, 2, 0, 1, 4)
        ).reshape(NT * P, 2, 2, P)
        in_maps.append({"xt": xh, "wu": wu8, "wv": wv8})

    results = None
    if os.environ.get("KERNEL_TRACE"):
        try:
            res = bass_utils.run_bass_kernel_spmd(
                _NC, in_maps, core_ids=list(range(N_CORES)), trace=True,
            )
            last_exec_time_ns = res.exec_time_ns
            results = res.results
        except Exception:
            results = None  # no NTFF profiling hook in this env; run untraced
    if results is None:
        results = _run_cached(_NC, in_maps)

    top = np.concatenate(
        [results[i]["top"] for i in range(N_CORES)], axis=0
    ).astype(np.float32)                                 # [N, FW] folded maxes

    # Expand fold-mates of every slot within MARGIN of the row max, then pick
    # by exact distance. Slot s covers codebook ids {s + FW*m}; all rows
    # sharing a slot rescore against the same 64 codebook rows, so the
    # rescore runs as <=FW small GEMMs instead of millions of gathered dots.
    in_margin = top >= (top.max(axis=1, keepdims=True) - MARGIN)
    W3 = np.ascontiguousarray(weight.reshape(FOLD, FW, D).transpose(1, 0, 2))
    c_sq = np.einsum("kd,kd->k", weight, weight)
    C3 = np.ascontiguousarray(c_sq.reshape(FOLD, FW).T).astype(np.float64)
    best = np.full(N, -1, dtype=np.int64)
    best_d = np.full(N, np.inf)
    for j in range(FW):
        rows = np.nonzero(in_margin[:, j])[0]
        if rows.size == 0:
            continue
        sc = x[rows] @ W3[j].T                     # [b, FOLD] fp32 GEMM
        d = C3[j][None, :] - 2.0 * sc.astype(np.float64)
        bi = np.argmin(d, axis=1)                  # first min -> smallest mate
        dmin = d[np.arange(len(rows)), bi]
        cmin = j + FW * bi
        sel = (dmin < best_d[rows]) | ((dmin == best_d[rows]) & (cmin < best[rows]))
        best[rows[sel]] = cmin[sel]
        best_d[rows[sel]] = dmin[sel]

    return weight[best]
